# revision 33
# baseline (speedup 1.0000x reference)
"""Distributed Trainium2 kernel for the cosine-similarity contrastive loss.

reference semantics (N=8192, D=256):
    qn = ||query_i||, rn = ||response_j||
    score[i,j] = (query_i . response_j) / max(qn_i * rn_j, 1e-8)
    loss = -(sum_i score[i,i] - logsumexp_j score[i,j])
         = sum_i (logsumexp_j score[i,j] - score[i,i])

Sharding: query rows are split across the 8 cores (1024 rows each); the
response matrix is replicated to every core; each core also receives the
1024 response rows matching its query rows (for the diagonal term).
Each core emits per-row (lse - pos) partials [128, 8]; the host sums them.

Per-core pipeline:
  1. DMA q [1024,256], rp [1024,256], r [8192,256] (f32, row-major).
  2. Row sumsq via DVE tensor_tensor_reduce; 1/norm = Exp(-0.5*Ln(sumsq))
     on ACT (keeps the whole kernel on one ACT table set: ln+exp).
  3. Normalize rows + cast bf16 (DVE tensor_scalar with per-partition AP).
  4. dma_start_transpose [128,128] blocks -> K-major qT/rT bf16.
  5. 8m x 16n x 2k matmuls (bf16, N=512) into [128,2048] PSUM tiles.
  6. ACT Exp with accum_out -> row sums of exp directly (no DVE reduce).
  7. Tail: sum 4 group partials, Log, subtract pos, DMA out [128,8].
"""

import numpy as np

N = 8192
D = 256
N_CORES = 8
ROWS_PER_CORE = N // N_CORES  # 1024
M_TILES = ROWS_PER_CORE // 128  # 8
R_TILES = N // 128  # 64
COHORTS = 4  # r processed in 4 cohorts of 2048 rows
TILES_PER_COHORT = R_TILES // COHORTS  # 16
COHORT_ROWS = N // COHORTS  # 2048
N_CHUNK = 512  # matmul moving free dim (one PSUM bank)
GROUP_COLS = 2048  # 4 banks per activation group
GROUPS = N // GROUP_COLS  # 4


def build(stage: int = 3):
    import concourse.bass as bass
    import concourse.bacc as bacc
    import concourse.tile as tile
    import concourse.mybir as mybir

    f32 = mybir.dt.float32
    bf16 = mybir.dt.bfloat16
    AF = mybir.ActivationFunctionType
    ALU = mybir.AluOpType

    nc = bacc.Bacc(None, target_bir_lowering=False, debug=False)

    q_d = nc.dram_tensor("q", [ROWS_PER_CORE, D], f32, kind="ExternalInput")
    rp_d = nc.dram_tensor("rp", [ROWS_PER_CORE, D], f32, kind="ExternalInput")
    r_d = nc.dram_tensor("r", [N, D], bf16, kind="ExternalInput")
    rt_d = nc.dram_tensor("rt", [D, N], bf16, kind="ExternalInput")
    qt_d = nc.dram_tensor("qt", [D, ROWS_PER_CORE], bf16, kind="ExternalInput")
    out_d = nc.dram_tensor("out", [128, 2 * M_TILES], f32, kind="ExternalOutput")

    with tile.TileContext(nc) as tc:
        with (
            tc.tile_pool(name="persist", bufs=1) as persist,
            tc.tile_pool(name="rchunk", bufs=8) as rchunk_pool,
            tc.tile_pool(name="sjunk", bufs=2) as sjunk_pool,
            tc.tile_pool(name="rt", bufs=1) as rt_pool,
            tc.tile_pool(name="psum", bufs=2, space=bass.MemorySpace.PSUM) as psum_pool,
            tc.tile_pool(name="dram", bufs=1, space=bass.MemorySpace.DRAM) as dram_pool,
        ):
            # ---------------- persistent buffers ----------------
            qbuf = persist.tile([128, M_TILES, D], f32)
            rpbuf = persist.tile([128, M_TILES, D], f32)
            qT = [persist.tile([128, ROWS_PER_CORE], bf16, name=f"qT{k}")
                  for k in range(2)]
            # rtraw: K-major unnormalized; rT: normalized (scaled in place-ish)
            rtraw = [[rt_pool.tile([128, GROUP_COLS], bf16, name=f"rtr{k}_{g}",
                                   tag=f"rtr{k}_{g}")
                      for g in range(COHORTS)] for k in range(2)]
            rT = [[rt_pool.tile([128, GROUP_COLS], bf16, name=f"rt{k}_{g}",
                                tag=f"rt{k}_{g}")
                   for g in range(COHORTS)] for k in range(2)]
            invb = [rt_pool.tile([128, GROUP_COLS], bf16, name=f"invb{g}",
                                 tag=f"invb{g}") for g in range(COHORTS)]
            invr_dram = [dram_pool.tile([GROUP_COLS], bf16, name=f"invrd{g}",
                                        tag=f"invrd{g}") for g in range(COHORTS)]
            rsq3 = persist.tile([128, R_TILES, 1], f32)
            qsq3 = persist.tile([128, M_TILES, 1], f32)
            rpsq3 = persist.tile([128, M_TILES, 1], f32)
            dot3 = persist.tile([128, M_TILES, 1], f32)
            inv_rn = persist.tile([128, R_TILES], f32)
            inv_rnb = persist.tile([128, R_TILES], bf16)
            inv_qn = persist.tile([128, M_TILES], f32)
            inv_rpn = persist.tile([128, M_TILES], f32)
            pos = persist.tile([128, M_TILES], f32)
            tmp8 = persist.tile([128, M_TILES], f32)
            acc = persist.tile([128, M_TILES, GROUPS], f32)
            se = persist.tile([128, M_TILES, 1], f32)
            lse = persist.tile([128, M_TILES], f32)
            diff = persist.tile([128, 2 * M_TILES], f32)

            def sumsq(src, acc_ap, junk):
                nc.vector.scalar_tensor_tensor(
                    out=junk, in0=src, scalar=1.0, in1=src,
                    op0=ALU.mult, op1=ALU.mult, accum_out=acc_ap,
                )

            i32 = mybir.dt.int32
            MAGIC = 0x5F3759DF

            def rsqrt(inv_ap, rsq_ap, nm, width, newton=2, final_dtype=None):
                magic = persist.tile([128, width], i32, name=f"mg_{nm}")
                nc.vector.memset(magic[:], MAGIC)
                sh = persist.tile([128, width], i32, name=f"sh_{nm}")
                nc.vector.tensor_scalar(
                    out=sh[:], in0=rsq_ap.bitcast(i32), scalar1=1,
                    scalar2=None, op0=ALU.arith_shift_right)
                y0i = persist.tile([128, width], i32, name=f"y0_{nm}")
                nc.vector.tensor_sub(y0i[:], magic[:], sh[:])
                y = y0i.bitcast(f32)[:]
                for it in range(newton):
                    aa = persist.tile([128, width], f32, name=f"nt{it}a_{nm}")
                    nc.vector.tensor_mul(aa[:], y, y)
                    bb = persist.tile([128, width], f32, name=f"nt{it}b_{nm}")
                    nc.vector.tensor_mul(bb[:], aa[:], rsq_ap)
                    cc = persist.tile([128, width], f32, name=f"nt{it}c_{nm}")
                    nc.vector.tensor_scalar(
                        out=cc[:], in0=bb[:], scalar1=-0.5, scalar2=1.5,
                        op0=ALU.mult, op1=ALU.add)
                    dt_ = (final_dtype if (final_dtype and it == newton - 1)
                           else f32)
                    yy = persist.tile([128, width], dt_, name=f"nt{it}y_{nm}")
                    nc.vector.tensor_mul(yy[:], y, cc[:])
                    y = yy[:]
                nc.vector.tensor_copy(inv_ap, y)

            # ---------------- loads: cohort-0 + qt first ----------
            for k in range(2):
                nc.sync.dma_start(qT[k][:], qt_d[k * 128 : (k + 1) * 128, :])
            all_chunks = []
            for a in range(2 * COHORTS):
                ch = rchunk_pool.tile([128, 8, D], bf16, tag="rch",
                                      name=f"rch{a}")
                all_chunks.append(ch)

            def load_chunk(a, eng=None):
                e = eng if eng is not None else nc.scalar
                e.dma_start(
                    all_chunks[a][:],
                    r_d[a * 1024 : (a + 1) * 1024, :].rearrange(
                        "(p t) d -> p t d", p=128
                    ),
                )

            load_chunk(0)
            load_chunk(1, eng=nc.sync)
            nc.scalar.dma_start(qbuf[:], q_d.rearrange("(p t) d -> p t d", p=128))


            # rt loads (after qT transposes on the sync ring)
            def load_rt(g):
                for k in range(2):
                    nc.sync.dma_start(
                        rtraw[k][g][:],
                        rt_d[k * 128 : (k + 1) * 128,
                             g * GROUP_COLS : (g + 1) * GROUP_COLS],
                    )

            load_rt(0)

            # ---------------- cohort prep: norms + K-major scale ----------
            def cohort_prep(g):
                chunks = all_chunks[g * 2 : g * 2 + 2]
                for jj in range(TILES_PER_COHORT):
                    t = g * TILES_PER_COHORT + jj
                    src = chunks[jj // 8][:, jj % 8, :]
                    jk = sjunk_pool.tile([128, D], bf16, tag="junk",
                                         name=f"jk{t}")
                    sumsq(src, rsq3[:, t, :], jk[:])
                sl = slice(g * TILES_PER_COHORT, (g + 1) * TILES_PER_COHORT)
                rsqrt(inv_rnb[:, sl], rsq3[:, sl, 0], f"r{g}", TILES_PER_COHORT,
                      newton=1, final_dtype=bf16)
                # scatter to DRAM in row order, gather back as a row
                nc.gpsimd.dma_start(
                    invr_dram[g].rearrange("(p w) -> p w", p=128),
                    inv_rnb[:, sl],
                )
                nc.gpsimd.dma_start(
                    invb[g][:],
                    invr_dram[g].rearrange("x -> () x").partition_broadcast(128),
                )
                for k in range(2):
                    nc.vector.tensor_mul(rT[k][g][:], rtraw[k][g][:], invb[g][:])

            def main_group(g):
                for m in range(M_TILES):
                    ps = psum_pool.tile([128, GROUP_COLS], f32, tag="ps",
                                        name=f"ps{g}_{m}")
                    for k in range(2):
                        for c in range(4):
                            nc.tensor.matmul(
                                ps[:, c * N_CHUNK : (c + 1) * N_CHUNK],
                                qT[k][:, m * 128 : (m + 1) * 128],
                                rT[k][g][:, c * N_CHUNK : (c + 1) * N_CHUNK],
                                start=(k == 0),
                                stop=(k == 1),
                            )
                    nc.scalar.activation(
                        ps[:], ps[:], AF.Exp,
                        scale=inv_qn[:, m : m + 1],
                        accum_out=acc[:, m, g : g + 1],
                    )

            with tc.tile_wait_until(0.010):
                load_chunk(2)
                load_chunk(3)
                load_rt(1)
            cohort_prep(0)

            # ---------------- q norms (needed only at first EXP) ---------
            for t in range(M_TILES):
                j = sjunk_pool.tile([128, D], f32, tag="junk")
                sumsq(qbuf[:, t, :], qsq3[:, t, :], j[:])
            rsqrt(inv_qn[:], qsq3[:, :, 0], "q", M_TILES)
            with tc.tile_wait_until(0.030):
                load_chunk(4)
                load_chunk(5)
                load_rt(2)
            if stage >= 3:
                main_group(0)
            cohort_prep(1)
            with tc.tile_wait_until(0.042):
                load_chunk(6)
                load_chunk(7)
                load_rt(3)
                nc.scalar.dma_start(
                    rpbuf[:], rp_d.rearrange("(p t) d -> p t d", p=128)
                )
            if stage >= 3:
                main_group(1)
            cohort_prep(2)
            if stage >= 3:
                main_group(2)
            cohort_prep(3)
            if stage >= 3:
                main_group(3)

            # ---------------- pos (diagonal) + tail ----------------
            for t in range(M_TILES):
                j1 = sjunk_pool.tile([128, D], f32, tag="junk")
                sumsq(rpbuf[:, t, :], rpsq3[:, t, :], j1[:])
                j2 = sjunk_pool.tile([128, D], f32, tag="junk")
                nc.vector.scalar_tensor_tensor(
                    out=j2[:], in0=qbuf[:, t, :], scalar=1.0, in1=rpbuf[:, t, :],
                    op0=ALU.mult, op1=ALU.mult, accum_out=dot3[:, t, :],
                )
            rsqrt(inv_rpn[:], rpsq3[:, :, 0], "rp", M_TILES)
            nc.vector.tensor_mul(tmp8[:], dot3[:, :, 0], inv_qn[:])
            nc.vector.tensor_mul(pos[:], tmp8[:], inv_rpn[:])

            if stage >= 3:
                nc.vector.tensor_reduce(
                    out=se[:], in_=acc[:], axis=mybir.AxisListType.X, op=ALU.add
                )
                nc.scalar.activation(lse[:], se[:, :, 0], AF.Ln)
            else:
                nc.vector.memset(lse[:], 0.0)
            nc.vector.tensor_copy(diff[:, 0:M_TILES], lse[:])
            nc.vector.tensor_copy(diff[:, M_TILES : 2 * M_TILES], pos[:])
            nc.scalar.dma_start(out_d[:], diff[:])

    nc.compile()
    return nc


_NC_CACHE = None


def _get_nc():
    global _NC_CACHE
    if _NC_CACHE is None:
        _NC_CACHE = build()
    return _NC_CACHE


def make_in_maps(query, response):
    import ml_dtypes

    query = np.ascontiguousarray(query, dtype=np.float32)
    response = np.ascontiguousarray(response, dtype=np.float32)
    assert query.shape == (N, D) and response.shape == (N, D)
    response_bf16 = np.ascontiguousarray(response.astype(ml_dtypes.bfloat16))
    # rt columns permuted per 2048-cohort: position p*16 + h*8 + t <- row h*1024 + 8p + t
    ridx = (np.arange(N).reshape(COHORTS, 2, 128, 8)
            .transpose(0, 2, 1, 3).reshape(-1))
    rt_bf16 = np.ascontiguousarray(response_bf16.T[:, ridx])
    query_bf16 = query.astype(ml_dtypes.bfloat16)
    # qt columns: position m*128 + p <- row 8p + m
    qidx = (np.arange(ROWS_PER_CORE).reshape(128, M_TILES).T.reshape(-1))
    in_maps = []
    for c in range(N_CORES):
        sl = slice(c * ROWS_PER_CORE, (c + 1) * ROWS_PER_CORE)
        in_maps.append(
            {"q": query[sl], "rp": np.ascontiguousarray(response[sl]),
             "r": response_bf16, "rt": rt_bf16,
             "qt": np.ascontiguousarray(query_bf16[sl].T[:, qidx])}
        )
    return in_maps


def kernel(query: np.ndarray, response: np.ndarray) -> np.ndarray:
    from concourse.bass_utils import run_bass_kernel_spmd

    in_maps = make_in_maps(query, response)
    nc = _get_nc()
    res = run_bass_kernel_spmd(nc, in_maps, core_ids=list(range(N_CORES)))
    total = 0.0
    for c in range(N_CORES):
        o = res.results[c]["out"].astype(np.float64)
        total += o[:, :M_TILES].sum() - o[:, M_TILES:].sum()
    return np.float32(total)


if __name__ == "__main__":
    rng = np.random.default_rng(0)
    q = rng.standard_normal((N, D), dtype=np.float32)
    r = rng.standard_normal((N, D), dtype=np.float32)
    print("loss:", kernel(q, r))
